# revision 11
# baseline (speedup 1.0000x reference)
"""Trainium2 Bass kernel for GQA multi-head attention with RoPE (causal).

Sharding (8 NeuronCores): 2-way data parallel over batch x 4-way sequence
parallel within each batch group.
  - core c: batch b = c//4, group rank j = c%4
  - KV: core computes K/V projections (+RoPE on K) for its contiguous 512-row
    chunk of the sequence, then AllGather over the 4-core group.
  - Q: core owns the strided query rows {j, j+4, j+8, ...} of its batch (512
    rows). Striding makes causal attention work identical on every core, so
    one SPMD program serves all 8 cores; causality enters only through
    host-supplied additive mask tables (per-core data).
  - Attention computed in transposed layout (scores^T = K^T-major) so softmax
    row sums come from N=1 matmuls and PV needs no transposes.
  - Output projection Wo is computed on the core's own query rows; host
    scatters rows back into the full (B, S, D) output. No output collective.

All matmuls run in bf16 with fp32 PSUM accumulation.
"""

import sys

sys.path.insert(0, "/opt/trn_rl_repo")

import numpy as np
import ml_dtypes

import concourse.bass as bass  # noqa: F401  (registers engine classes)
import concourse.bacc as bacc
import concourse.mybir as mybir
import concourse.tile as tile
from concourse.bass_utils import run_bass_kernel_spmd

BF16 = ml_dtypes.bfloat16

B, S, D = 2, 2048, 4096
H, KVH, DH = 32, 8, 128
ROPE_BASE = 10000.0
NCORES, TPG = 8, 4          # total cores, cores per batch group
KVC = S // TPG              # 512: kv rows per core
TQ = S // TPG               # 512: query rows per core
KC = D // 128               # 32: contraction chunks of 128
KT = S // 128               # 16: kv tiles per batch
NPAIR = 2                   # query processed as 2 pairs of 128-row tiles
NEG = -1.0e9
SCALE = 1.0 / float(np.sqrt(DH))
F32 = mybir.dt.float32
BF = mybir.dt.bfloat16
GROUPS = [[0, 1, 2, 3], [4, 5, 6, 7]]

_NC = None


def _rope(nc, tmp_pool, ps, cos_sb, sin_sb, out_bf):
    """RoPE in [dh, t] layout: out = ps*cos + rotate_half(ps)*sin, bf16 out."""
    T = ps.shape[-1]
    tcos = tmp_pool.tile([128, T], F32, tag="rope_c")
    tsin = tmp_pool.tile([128, T], F32, tag="rope_s")
    nc.vector.tensor_mul(tcos[:], ps[:], cos_sb[:])
    nc.vector.tensor_mul(tsin[0:64, :], ps[64:128, :], sin_sb[0:64, :])
    nc.vector.tensor_mul(tsin[64:128, :], ps[0:64, :], sin_sb[64:128, :])
    nc.vector.tensor_sub(out_bf[0:64, :], tcos[0:64, :], tsin[0:64, :])
    nc.vector.tensor_add(out_bf[64:128, :], tcos[64:128, :], tsin[64:128, :])


def _build(sim_single_core=False, max_phase=4):
    nd = 1 if sim_single_core else NCORES
    nc = bacc.Bacc("TRN2", target_bir_lowering=False, debug=False, num_devices=nd)

    xq = nc.declare_dram_parameter("xq", [D, TQ], BF, isOutput=False)
    xkv = nc.declare_dram_parameter("xkv", [D, KVC], BF, isOutput=False)
    wq = nc.declare_dram_parameter("wq", [D, D], BF, isOutput=False)
    wk = nc.declare_dram_parameter("wk", [D, KVH * DH], BF, isOutput=False)
    wv = nc.declare_dram_parameter("wv", [D, KVH * DH], BF, isOutput=False)
    wo = nc.declare_dram_parameter("wo", [D, D], BF, isOutput=False)
    bo = nc.declare_dram_parameter("bo", [D], F32, isOutput=False)
    cos_q = nc.declare_dram_parameter("cos_q", [DH, TQ], F32, isOutput=False)
    sin_q = nc.declare_dram_parameter("sin_q", [DH, TQ], F32, isOutput=False)
    cos_kv = nc.declare_dram_parameter("cos_kv", [DH, KVC], F32, isOutput=False)
    sin_kv = nc.declare_dram_parameter("sin_kv", [DH, KVC], F32, isOutput=False)
    dmask = nc.declare_dram_parameter("dmask", [128, 8, 256], F32, isOutput=False)
    out = nc.declare_dram_parameter("out", [TQ, D], F32, isOutput=True)

    k_sh = nc.dram_tensor("k_sh", [KVH, DH, KVC], BF)
    v_sh = nc.dram_tensor("v_sh", [KVC, KVH * DH], BF)
    k_g = nc.dram_tensor("k_g", [TPG, KVH, DH, KVC], BF)
    v_g = nc.dram_tensor("v_g", [TPG, KVC, KVH * DH], BF)

    with tile.TileContext(nc) as tc:
        with tc.tile_pool(name="const", bufs=1) as const:
            ones = const.tile([128, 1], BF)
            nc.vector.memset(ones[:], 1.0)

            # ---- Phase 1: KV projection + RoPE(K) + AllGather ----
            with (
                tc.tile_pool(name="p1x", bufs=1) as p1x,
                tc.tile_pool(name="p1w", bufs=1) as p1w,
                tc.tile_pool(name="p1k", bufs=2) as p1k,
                tc.tile_pool(name="p1o", bufs=3) as p1o,
                tc.tile_pool(name="p1t", bufs=2) as p1t,
                tc.tile_pool(name="p1ps", bufs=2, space="PSUM") as p1ps,
            ):
                xkv_sb = p1x.tile([128, KC, KVC], BF)
                nc.sync.dma_start(
                    xkv_sb[:], xkv.rearrange("(kc p) t -> p kc t", p=128)
                )
                cos_kv_sb = p1x.tile([128, KVC], F32)
                sin_kv_sb = p1x.tile([128, KVC], F32)
                nc.sync.dma_start(cos_kv_sb[:], cos_kv[:])
                nc.sync.dma_start(sin_kv_sb[:], sin_kv[:])
                wv_sb = p1w.tile([128, KC, KVH * DH], BF)
                nc.sync.dma_start(wv_sb[:], wv.rearrange("(kc p) c -> p kc c", p=128))

                for kvh in range(KVH):
                    wk_h = p1k.tile([128, KC, DH], BF, tag="wk_h")
                    nc.sync.dma_start(
                        wk_h[:],
                        wk[:, kvh * DH : (kvh + 1) * DH].rearrange(
                            "(kc p) c -> p kc c", p=128
                        ),
                    )
                    ps = p1ps.tile([128, KVC], F32, tag="p1ps")
                    for kc in range(KC):
                        nc.tensor.matmul(
                            ps[:], wk_h[:, kc], xkv_sb[:, kc],
                            start=(kc == 0), stop=(kc == KC - 1),
                        )
                    k_out = p1o.tile([128, KVC], BF, tag="k_out")
                    _rope(nc, p1t, ps, cos_kv_sb, sin_kv_sb, k_out)
                    nc.sync.dma_start(k_sh[kvh], k_out[:])

                for t4 in range(KVC // 128):
                    for nn in range(2):
                        ps = p1ps.tile([128, 512], F32, tag="p1ps")
                        for kc in range(KC):
                            nc.tensor.matmul(
                                ps[:],
                                xkv_sb[:, kc, t4 * 128 : (t4 + 1) * 128],
                                wv_sb[:, kc, nn * 512 : (nn + 1) * 512],
                                start=(kc == 0), stop=(kc == KC - 1),
                            )
                        v_out = p1o.tile([128, 512], BF, tag="v_out")
                        nc.vector.tensor_copy(v_out[:], ps[:])
                        nc.sync.dma_start(
                            v_sh[t4 * 128 : (t4 + 1) * 128, nn * 512 : (nn + 1) * 512],
                            v_out[:],
                        )

                if sim_single_core:
                    # stand-in for the AllGather with equivalent-size DMA
                    # traffic so TimelineSim (single-core, no collectives)
                    # can model the kernel
                    for g in range(TPG):
                        nc.sync.dma_start(k_g[g], k_sh[:])
                        nc.sync.dma_start(v_g[g], v_sh[:])
                else:
                    nc.gpsimd.collective_compute(
                        "AllGather", mybir.AluOpType.bypass,
                        replica_groups=GROUPS, ins=[k_sh[:]], outs=[k_g[:]],
                    )
                    nc.gpsimd.collective_compute(
                        "AllGather", mybir.AluOpType.bypass,
                        replica_groups=GROUPS, ins=[v_sh[:]], outs=[v_g[:]],
                    )

            with tc.tile_pool(name="qt", bufs=1) as qtp:
                qT = qtp.tile([128, H, TQ], BF)

                # ---- Phase 2: Q projection + RoPE ----
                with (
                    tc.tile_pool(name="p2x", bufs=1) as p2x,
                    tc.tile_pool(name="p2w", bufs=2) as p2w,
                    tc.tile_pool(name="p2t", bufs=2) as p2t,
                    tc.tile_pool(name="p2ps", bufs=2, space="PSUM") as p2ps,
                ):
                    xq_sb = p2x.tile([128, KC, TQ], BF)
                    nc.sync.dma_start(
                        xq_sb[:], xq.rearrange("(kc p) t -> p kc t", p=128)
                    )
                    cos_q_sb = p2x.tile([128, TQ], F32)
                    sin_q_sb = p2x.tile([128, TQ], F32)
                    nc.sync.dma_start(cos_q_sb[:], cos_q[:])
                    nc.sync.dma_start(sin_q_sb[:], sin_q[:])

                    for h in range(H if max_phase >= 2 else 0):
                        wq_h = p2w.tile([128, KC, DH], BF, tag="wq_h")
                        nc.sync.dma_start(
                            wq_h[:],
                            wq[:, h * DH : (h + 1) * DH].rearrange(
                                "(kc p) c -> p kc c", p=128
                            ),
                        )
                        ps = p2ps.tile([128, TQ], F32, tag="p2ps")
                        for kc in range(KC):
                            nc.tensor.matmul(
                                ps[:], wq_h[:, kc], xq_sb[:, kc],
                                start=(kc == 0), stop=(kc == KC - 1),
                            )
                        _rope(nc, p2t, ps, cos_q_sb, sin_q_sb, qT[:, h])

                # ---- Phase 3+4: attention, with the output projection of
                # each 8-head group overlapped with the next group's
                # attention (partials accumulate into `out` via DMA-add) ----
                HG = H // 4
                with (
                    tc.tile_pool(name="kvsb", bufs=1) as kvp,
                    tc.tile_pool(name="attng", bufs=2) as attnp,
                    tc.tile_pool(name="pt", bufs=2) as ptp,
                    tc.tile_pool(name="nrm", bufs=2) as nrm,
                    tc.tile_pool(name="msk", bufs=1) as mskp,
                    tc.tile_pool(name="p4w", bufs=2) as p4w,
                    tc.tile_pool(name="p4b", bufs=1) as p4b,
                    tc.tile_pool(name="p4o", bufs=3) as p4o,
                    tc.tile_pool(name="psS", bufs=2, space="PSUM") as psS,
                    tc.tile_pool(name="psPV", bufs=2, space="PSUM") as psPV,
                    tc.tile_pool(name="psSum", bufs=2, space="PSUM") as psSum,
                    tc.tile_pool(name="p4ps", bufs=2, space="PSUM") as p4ps,
                ):
                    k_sb = kvp.tile([128, KVH, TPG, KVC], BF)
                    for g in range(TPG):
                        for kvh in range(KVH):
                            nc.sync.dma_start(k_sb[:, kvh, g], k_g[g, kvh])
                    v_sb = kvp.tile([128, KT, KVH * DH], BF)
                    for kt in range(KT):
                        nc.sync.dma_start(
                            v_sb[:, kt],
                            v_g[kt // 4, (kt % 4) * 128 : (kt % 4 + 1) * 128, :],
                        )
                    dm_sb = mskp.tile([128, 8, 256], F32)
                    nc.sync.dma_start(dm_sb[:], dmask[:])
                    bo_sb = p4b.tile([128, D], F32)
                    nc.sync.dma_start(
                        bo_sb[:], bo[:][None, :].to_broadcast((128, D))
                    )

                    for g in range(4 if max_phase >= 3 else 0):
                        attnTg = attnp.tile([128, HG, TQ], BF, tag="attnTg")
                        for hh in range(HG):
                            h = g * HG + hh
                            kvh = h // (H // KVH)
                            for p in range(NPAIR):
                                n_kt = 8 * p + 8
                                pT = ptp.tile([128, KT * 256], BF, tag="pT")
                                pv = psPV.tile([128, 256], F32, tag="pv")
                                srow = psSum.tile([1, 256], F32, tag="srow")
                                for k2 in range(n_kt // 2):
                                    # two kv tiles share one PSUM bank /
                                    # one mask add / one exp
                                    sT = psS.tile([128, 512], F32, tag="sT")
                                    for u in range(2):
                                        kt = 2 * k2 + u
                                        nc.tensor.matmul(
                                            sT[:, u * 256 : (u + 1) * 256],
                                            k_sb[:, kvh, kt // 4,
                                                 (kt % 4) * 128 : (kt % 4 + 1) * 128],
                                            qT[:, h, p * 256 : (p + 1) * 256],
                                            start=(u == 0), stop=(u == 1),
                                        )
                                    r = 2 * k2 - 8 * p
                                    if r >= 0:
                                        nc.vector.tensor_add(
                                            sT[:], sT[:],
                                            dm_sb[:, r : r + 2, :],
                                        )
                                    nc.scalar.activation(
                                        pT[:, 2 * k2 * 256 : (2 * k2 + 2) * 256],
                                        sT[:],
                                        mybir.ActivationFunctionType.Exp,
                                        scale=SCALE,
                                    )
                                    for u in range(2):
                                        kt = 2 * k2 + u
                                        psl = pT[:, kt * 256 : (kt + 1) * 256]
                                        nc.tensor.matmul(
                                            srow[:], ones[:], psl,
                                            start=(kt == 0), stop=(kt == n_kt - 1),
                                        )
                                        nc.tensor.matmul(
                                            pv[:],
                                            v_sb[:, kt, kvh * DH : (kvh + 1) * DH],
                                            psl,
                                            start=(kt == 0), stop=(kt == n_kt - 1),
                                        )
                                recip = nrm.tile([1, 256], F32, tag="recip")
                                nc.vector.reciprocal(recip[:], srow[:])
                                bc = nrm.tile([128, 256], F32, tag="bc")
                                nc.gpsimd.partition_broadcast(bc[:], recip[:])
                                nc.vector.tensor_mul(
                                    attnTg[:, hh, p * 256 : (p + 1) * 256],
                                    pv[:], bc[:],
                                )

                        for nn in range((D // 512) if max_phase >= 4 else 0):
                            wo_g = p4w.tile([128, HG, 512], BF, tag="wo_g")
                            nc.sync.dma_start(
                                wo_g[:],
                                wo[g * HG * 128 : (g + 1) * HG * 128,
                                   nn * 512 : (nn + 1) * 512].rearrange(
                                    "(h p) c -> p h c", p=128
                                ),
                            )
                            for tq in range(TQ // 128):
                                ps = p4ps.tile([128, 512], F32, tag="p4ps")
                                for hh in range(HG):
                                    nc.tensor.matmul(
                                        ps[:],
                                        attnTg[:, hh, tq * 128 : (tq + 1) * 128],
                                        wo_g[:, hh],
                                        start=(hh == 0), stop=(hh == HG - 1),
                                    )
                                osb = p4o.tile([128, 512], F32, tag="osb")
                                oslice = out[tq * 128 : (tq + 1) * 128,
                                             nn * 512 : (nn + 1) * 512]
                                if g == 0:
                                    nc.vector.tensor_add(
                                        osb[:], ps[:],
                                        bo_sb[:, nn * 512 : (nn + 1) * 512],
                                    )
                                    nc.sync.dma_start(oslice, osb[:])
                                else:
                                    nc.vector.tensor_copy(osb[:], ps[:])
                                    nc.gpsimd.dma_start(
                                        oslice, osb[:],
                                        accum_op=mybir.AluOpType.add,
                                    )

    nc.compile()
    return nc


def _get_nc():
    global _NC
    if _NC is None:
        _NC = _build()
    return _NC


def _rope_tables_T(positions):
    """cos/sin tables in [DH, T] layout for given absolute positions."""
    inv_freq = 1.0 / (ROPE_BASE ** (np.arange(0, DH, 2, dtype=np.float64) / DH))
    freqs = inv_freq[:, None] * positions[None, :].astype(np.float64)  # (64, T)
    emb = np.concatenate([freqs, freqs], axis=0)  # (128, T)
    return np.cos(emb).astype(np.float32), np.sin(emb).astype(np.float32)


def _diag_masks(j):
    """Additive mask table [128 kv, 8 rel-tiles, 256 q] for group rank j."""
    i = np.arange(128)
    jj = np.arange(128)
    m = np.full((128, 8, 256), NEG, dtype=np.float32)
    for r in range(8):
        kvpos = 128 * r + jj[:, None]           # (128, 1)
        lo = kvpos <= 4 * i[None, :] + j        # (128, 128)
        hi = kvpos <= 512 + 4 * i[None, :] + j
        m[:, r, 0:128][lo] = 0.0
        m[:, r, 128:256][hi] = 0.0
    return m


def make_in_maps(x, Wq, Wk, Wv, Wo, bo):
    wq_bf = Wq.astype(BF16)
    wk_bf = Wk.astype(BF16)
    wv_bf = Wv.astype(BF16)
    wo_bf = Wo.astype(BF16)
    bo_f = bo.astype(np.float32)
    in_maps = []
    for c in range(NCORES):
        b, j = divmod(c, TPG)
        qpos = np.arange(j, S, TPG)
        kvpos = np.arange(j * KVC, (j + 1) * KVC)
        cq, sq = _rope_tables_T(qpos)
        ckv, skv = _rope_tables_T(kvpos)
        in_maps.append({
            "xq": np.ascontiguousarray(x[b, qpos, :].T).astype(BF16),
            "xkv": np.ascontiguousarray(x[b, kvpos, :].T).astype(BF16),
            "wq": wq_bf, "wk": wk_bf, "wv": wv_bf, "wo": wo_bf, "bo": bo_f,
            "cos_q": cq, "sin_q": sq, "cos_kv": ckv, "sin_kv": skv,
            "dmask": _diag_masks(j),
        })
    return in_maps


def assemble_output(results):
    out = np.empty((B, S, D), dtype=np.float32)
    for c in range(NCORES):
        b, j = divmod(c, TPG)
        out[b, j::TPG, :] = results[c]["out"]
    return out


def kernel(x, Wq, Wk, Wv, Wo, bo):
    nc = _get_nc()
    in_maps = make_in_maps(
        np.asarray(x, dtype=np.float32), np.asarray(Wq), np.asarray(Wk),
        np.asarray(Wv), np.asarray(Wo), np.asarray(bo),
    )
    res = run_bass_kernel_spmd(nc, in_maps, list(range(NCORES)))
    return assemble_output(res.results)
